# revision 1
# baseline (speedup 1.0000x reference)
"""Trainium2 Bass kernel for nn_BiMambaBlock (B=2, L=1024, d_model=512).

Strategy (8 NeuronCores, SPMD — one identical program, per-core data):
  core c = (b, dir, half) with slot index c = b*4 + dir*2 + half.
  - dir is handled by feeding bwd cores time-flipped x; the whole Mamba
    pipeline runs in "physical" (possibly flipped) time. A data-driven
    blend (alpha,beta in {0,1} per core) un-flips the gated output g for
    bwd cores, so the program has zero direction-dependent control flow.
  - Channel halves: the host permutes the in-proj weight columns so the
    core's OWN 512 channels are always u-blocks 0..3; matching row
    permutations are applied to W_xproj / conv weights.
  - Each core computes: rmsnorm -> in-proj (full u for x_dbl, own z half)
    -> causal conv -> x_dbl -> dt/B/C -> selective scan on its 512
    channels -> gated g -> un-flip blend -> 8-way AllToAll (token
    sharding, bf16) -> tail (out-proj, GLU fuse, FF, residual, out-norm)
    for its 128-token slice of BOTH batches.
Scan: channels in partitions, time in the free dim. n-outer loop over
the 64 state indices: dA = exp(a_n*dt) on ScalarE (a_n = per-partition
scale column), B/C rows broadcast across partitions by GPSIMD
partition_broadcast from a single-partition flat copy, recurrence via
the DVE tensor_tensor_scan instruction (fp32 state), y accumulated over
n with bf16 identity matmuls into PSUM (fp32).
Matmuls run in bf16 (weights pre-cast on host); the scan decay path
(dt, dA, scan state) stays fp32.
"""

import numpy as np

D_MODEL = 512
D_STATE = 64
D_CONV = 4
D_INNER = 1024
DT_RANK = 32
B = 2
L = 1024
EPS = 1e-6
NCORES = 8
CH = 512
TOK = L // NCORES

_CACHE = {}


def _build(single_core=False, skip_scan=False, skip_comm=False,
           fake_act=False):
    import concourse.bacc as bacc
    import concourse.mybir as mybir
    import concourse.tile as tile
    from concourse.masks import make_identity

    f32 = mybir.dt.float32
    bf16 = mybir.dt.bfloat16
    AF = mybir.ActivationFunctionType
    OP = mybir.AluOpType
    if fake_act:
        class _FA:
            Square = AF.Square
            Ln = AF.Square
            Exp = AF.Square
            Silu = AF.Square
            Sigmoid = AF.Square
        AF = _FA

    nc = bacc.Bacc("TRN2", target_bir_lowering=False, debug=False,
                   num_devices=1 if single_core else NCORES)

    def din(name, shape, dt_=f32):
        return nc.dram_tensor(name, shape, dt_, kind="ExternalInput")

    xb = din("xb", [L, D_MODEL])
    x_res = din("x_res", [2 * TOK, D_MODEL])
    w_in_T = din("w_in_T", [D_MODEL, D_INNER + CH], bf16)
    convw = din("convw", [128, 8 * D_CONV])
    convb = din("convb", [128, 8])
    w_xproj_T = din("w_xproj_T", [D_INNER, DT_RANK + 2 * D_STATE], bf16)
    w_dt_T = din("w_dt_T", [DT_RANK, CH], bf16)
    b_dt_col = din("b_dt_col", [128, 4])
    dskip_col = din("dskip_col", [128, 4])
    ab_cols = din("ab_cols", [128, 2])
    a_rep = din("a_rep", [128, D_STATE])
    ident_bf = din("ident_bf", [128, 128], bf16)
    w_out_T = din("w_out_T", [D_INNER, D_MODEL], bf16)
    fuse_w_T = din("fuse_w_T", [2 * D_MODEL, 2 * D_MODEL], bf16)
    fuse_b_col = din("fuse_b_col", [128, 8])
    ff1_T = din("ff1_T", [D_MODEL, 4 * D_MODEL], bf16)
    ff2_T = din("ff2_T", [4 * D_MODEL, D_MODEL], bf16)
    w_nout_rep = din("w_nout_rep", [128, D_MODEL])
    out = nc.dram_tensor("out", [2 * TOK, D_MODEL], f32, kind="ExternalOutput")

    with tile.TileContext(nc) as tc:
        with (
            tc.tile_pool(name="wpool", bufs=1) as wp,
            tc.tile_pool(name="actp", bufs=1) as actp,
            tc.tile_pool(name="dram", bufs=1, space="DRAM") as dramp,
        ):
            c_eps = wp.tile([128, 1], f32, name="c_eps")
            nc.vector.memset(c_eps[:], EPS)
            c_one = wp.tile([128, 1], f32, name="c_one")
            nc.vector.memset(c_one[:], 1.0)
            ident = wp.tile([128, 128], bf16, name="ident")
            nc.sync.dma_start(ident[:], ident_bf.ap())
            idf = wp.tile([128, 128], f32, name="idf")
            make_identity(nc, idf[:])
            convw_sb = wp.tile([128, 8 * D_CONV], f32, name="convw_sb")
            nc.sync.dma_start(convw_sb[:], convw.ap())
            convb_sb = wp.tile([128, 8], f32, name="convb_sb")
            nc.sync.dma_start(convb_sb[:], convb.ap())
            bdt_sb = wp.tile([128, 4], f32, name="bdt_sb")
            nc.sync.dma_start(bdt_sb[:], b_dt_col.ap())
            dskip_sb = wp.tile([128, 4], f32, name="dskip_sb")
            nc.sync.dma_start(dskip_sb[:], dskip_col.ap())
            ab_sb = wp.tile([128, 2], f32, name="ab_sb")
            nc.sync.dma_start(ab_sb[:], ab_cols.ap())
            arep_sb = wp.tile([128, D_STATE], f32, name="arep_sb")
            nc.sync.dma_start(arep_sb[:], a_rep.ap())

            send = dramp.tile([NCORES * CH, TOK], bf16, name="sendbuf")
            recv = dramp.tile([NCORES * CH, TOK], bf16, name="recvbuf")
            bc_dram = dramp.tile([D_STATE, 2 * L], bf16, name="bc_dram")

            g_send = [actp.tile([128, L], bf16, name=f"gs{i}", tag=f"gs{i}")
                      for i in range(4)]

            # =========== Phase A / B (scan-lifetime pool) ===========
            ctx_scanp = tc.tile_pool(name="scanp", bufs=1)
            scanp = ctx_scanp.__enter__()
            z_raw = [scanp.tile([128, L], f32, name=f"z{i}", tag=f"z{i}")
                     for i in range(4)]
            dtw = [scanp.tile([128, 2 * L], f32, name=f"dtw{i}", tag=f"dtw{i}")
                   for i in range(2)]
            dtu_bf = [scanp.tile([128, 2 * L], bf16, name=f"dtu{i}",
                                 tag=f"dtu{i}") for i in range(2)]
            u_own = [scanp.tile([128, L], f32, name=f"uo{i}", tag=f"uo{i}")
                     for i in range(4)]

            with (
                tc.tile_pool(name="uop", bufs=1) as uop,
                tc.tile_pool(name="wxw", bufs=1) as wxw,
                tc.tile_pool(name="pa", bufs=2) as pa,
                tc.tile_pool(name="pa_ps", bufs=2, space="PSUM") as pa_ps,
                tc.tile_pool(name="upp", bufs=1) as upp,
            ):
                wxp_sb = [wxw.tile([128, 160], bf16, name=f"wx{k}", tag=f"wx{k}")
                          for k in range(8)]
                for kt in range(8):
                    nc.sync.dma_start(wxp_sb[kt][:],
                                      w_xproj_T.ap()[kt * 128:(kt + 1) * 128, :])
                wdt_sb = wxw.tile([DT_RANK, CH], bf16, name="wdt_sb")
                nc.sync.dma_start(wdt_sb[:], w_dt_T.ap())
                dtr_sb = wxw.tile([32, L], bf16, name="dtr_sb")
                B_sb = wxw.tile([64, L], bf16, name="B_sb")
                C_sb = wxw.tile([64, L], bf16, name="C_sb")

                ip_ctx = tc.tile_pool(name="ipw", bufs=1)
                ipw = ip_ctx.__enter__()
                winT_sb = [ipw.tile([128, D_INNER + CH], bf16,
                                    name=f"wi{k}", tag=f"wi{k}")
                           for k in range(4)]
                for kt in range(4):
                    nc.sync.dma_start(winT_sb[kt][:],
                                      w_in_T.ap()[kt * 128:(kt + 1) * 128, :])

                # rmsnorm + transpose -> hT (bf16) [4][128, L]
                hT = [ipw.tile([128, L], bf16, name=f"hT{i}", tag=f"hT{i}")
                      for i in range(4)]
                for tb in range(8):
                    xt = pa.tile([128, D_MODEL], f32, name="xt", tag="xt")
                    nc.sync.dma_start(xt[:], xb.ap()[tb * 128:(tb + 1) * 128, :])
                    hn = pa.tile([128, D_MODEL], f32, name="hn", tag="hn")
                    ssum = pa.tile([128, 1], f32, name="ssum", tag="ssum")
                    nc.scalar.activation(hn[:], xt[:], AF.Square,
                                         accum_out=ssum[:])
                    lnv = pa.tile([128, 1], f32, name="lnv", tag="lnv")
                    nc.scalar.activation(lnv[:], ssum[:], AF.Ln,
                                         scale=1.0 / D_MODEL, bias=c_eps[:])
                    rinv = pa.tile([128, 1], f32, name="rinv", tag="rinv")
                    nc.scalar.activation(rinv[:], lnv[:], AF.Exp, scale=-0.5)
                    nc.vector.tensor_scalar(hn[:], xt[:], rinv[:], None,
                                            OP.mult)
                    for db in range(4):
                        tp = pa_ps.tile([128, 128], f32, name="tp", tag="tp")
                        nc.tensor.transpose(tp[:],
                                            hn[:, db * 128:(db + 1) * 128],
                                            idf[:])
                        nc.vector.tensor_copy(
                            hT[db][:, tb * 128:(tb + 1) * 128], tp[:])

                # in-proj -> u_pre (bf16, full Din) and z_silu (own half)
                u_pre = [upp.tile([128, L], bf16, name=f"up{i}", tag=f"up{i}")
                         for i in range(8)]
                for mb in range(12):
                    for nh in range(2):
                        ps = pa_ps.tile([128, 512], f32, name="mm", tag="mm")
                        for kt in range(4):
                            nc.tensor.matmul(
                                ps[:],
                                winT_sb[kt][:, mb * 128:(mb + 1) * 128],
                                hT[kt][:, nh * 512:(nh + 1) * 512],
                                start=(kt == 0), stop=(kt == 3))
                        if mb < 8:
                            nc.vector.tensor_copy(
                                u_pre[mb][:, nh * 512:(nh + 1) * 512], ps[:])
                        else:
                            zb = mb - 8
                            nc.vector.tensor_copy(
                                z_raw[zb][:, nh * 512:(nh + 1) * 512], ps[:])
                ip_ctx.__exit__(None, None, None)

                # causal conv (zero left pad via shrinking write ranges)
                u_bf = [uop.tile([128, L], bf16, name=f"ub{i}", tag=f"ub{i}")
                        for i in range(8)]
                KC = D_CONV - 1
                for cb in range(8):
                    uc = pa.tile([128, L], f32, name="uc", tag="uc", bufs=1)
                    nc.vector.tensor_scalar(
                        uc[:], u_pre[cb][:],
                        convw_sb[:, cb * 4 + KC:cb * 4 + KC + 1], None,
                        OP.mult)
                    for k in range(KC):
                        sh = KC - k
                        nc.vector.scalar_tensor_tensor(
                            uc[:, sh:L], u_pre[cb][:, 0:L - sh],
                            convw_sb[:, cb * 4 + k:cb * 4 + k + 1],
                            uc[:, sh:L], OP.mult, OP.add)
                    if cb < 4:
                        nc.scalar.activation(u_own[cb][:], uc[:], AF.Silu,
                                             bias=convb_sb[:, cb:cb + 1])
                        nc.vector.tensor_copy(u_bf[cb][:], u_own[cb][:])
                    else:
                        nc.scalar.activation(u_bf[cb][:], uc[:], AF.Silu,
                                             bias=convb_sb[:, cb:cb + 1])

                # x_dbl -> dtr (bf16), B, C (fp32)
                for nh in range(2):
                    ps0 = pa_ps.tile([32, 512], f32, name="mm32", tag="mm32",
                                     bufs=1)
                    ps1 = pa_ps.tile([64, 512], f32, name="mmB", tag="mmB",
                                     bufs=1)
                    ps2 = pa_ps.tile([64, 512], f32, name="mmC", tag="mmC",
                                     bufs=1)
                    for kt in range(8):
                        nc.tensor.matmul(
                            ps0[:], wxp_sb[kt][:, 0:32],
                            u_bf[kt][:, nh * 512:(nh + 1) * 512],
                            start=(kt == 0), stop=(kt == 7))
                    for kt in range(8):
                        nc.tensor.matmul(
                            ps1[:], wxp_sb[kt][:, 32:96],
                            u_bf[kt][:, nh * 512:(nh + 1) * 512],
                            start=(kt == 0), stop=(kt == 7))
                    for kt in range(8):
                        nc.tensor.matmul(
                            ps2[:], wxp_sb[kt][:, 96:160],
                            u_bf[kt][:, nh * 512:(nh + 1) * 512],
                            start=(kt == 0), stop=(kt == 7))
                    nc.vector.tensor_copy(dtr_sb[:, nh * 512:(nh + 1) * 512],
                                          ps0[:])
                    nc.vector.tensor_copy(B_sb[:, nh * 512:(nh + 1) * 512],
                                          ps1[:])
                    nc.vector.tensor_copy(C_sb[:, nh * 512:(nh + 1) * 512],
                                          ps2[:])

                # B/C -> interleaved [n, B_n|C_n] DRAM bounce rows
                nc.sync.dma_start(bc_dram[:, 0:L], B_sb[:])
                nc.sync.dma_start(bc_dram[:, L:2 * L], C_sb[:])

                # dt = softplus(dtr @ W_dt^T + b_dt); dtu = dt * u_own
                for mb in range(4):
                    for nh in range(2):
                        ps = pa_ps.tile([128, 512], f32, name="mm", tag="mm")
                        nc.tensor.matmul(
                            ps[:], wdt_sb[:, mb * 128:(mb + 1) * 128],
                            dtr_sb[:, nh * 512:(nh + 1) * 512],
                            start=True, stop=True)
                        ex = pa.tile([128, 512], f32, name="ex", tag="ex")
                        nc.scalar.activation(ex[:], ps[:], AF.Exp,
                                             bias=bdt_sb[:, mb:mb + 1])
                        off = (mb % 2) * L + nh * 512
                        nc.scalar.activation(
                            dtw[mb // 2][:, off:off + 512],
                            ex[:], AF.Ln, bias=c_one[:])
                for cb in range(4):
                    p_, hh = cb // 2, cb % 2
                    nc.vector.tensor_tensor(
                        dtu_bf[p_][:, hh * L:(hh + 1) * L],
                        dtw[p_][:, hh * L:(hh + 1) * L],
                        u_own[cb][:], OP.mult)

            # =========== Phase B: the scan (n outer) ===========
            with (
                tc.tile_pool(name="sb_ps", bufs=1, space="PSUM") as sb_ps,
                tc.tile_pool(name="sbl", bufs=2) as sbl,
            ):
                y_ps = [sb_ps.tile([128, L], f32, name=f"y{cb}", tag=f"y{cb}")
                        for cb in range(4)]
                n_states = 1 if skip_scan else D_STATE
                for n in range(n_states):
                    bcrow = sbl.tile([1, 2 * L], bf16, name="bcrow",
                                     tag="bcrow")
                    nc.sync.dma_start(bcrow[:], bc_dram[n:n + 1, :])
                    BC = sbl.tile([128, 2 * L], bf16, name="BC", tag="BC")
                    nc.gpsimd.partition_broadcast(BC[:], bcrow[:])
                    for p_ in range(2):
                        dA = sbl.tile([128, 2 * L], f32, name="dA", tag="dA")
                        nc.scalar.activation(dA[:], dtw[p_][:], AF.Exp,
                                             scale=arep_sb[:, n:n + 1])
                        dBu = sbl.tile([128, 2 * L], bf16, name="dBu",
                                       tag="dBu")
                        nc.vector.tensor_tensor(
                            dBu[:].rearrange("p (a t) -> p a t", a=2),
                            dtu_bf[p_][:].rearrange("p (a t) -> p a t", a=2),
                            BC[:, 0:L].unsqueeze(1).broadcast_to((128, 2, L)),
                            OP.mult)
                        s_w = sbl.tile([128, 2 * L], bf16, name="s_w",
                                       tag="s_w")
                        for hh in range(2):
                            nc.vector.tensor_tensor_scan(
                                s_w[:, hh * L:(hh + 1) * L],
                                dA[:, hh * L:(hh + 1) * L],
                                dBu[:, hh * L:(hh + 1) * L],
                                0.0, OP.mult, OP.add)
                        P = sbl.tile([128, 2 * L], bf16, name="P", tag="P")
                        nc.vector.tensor_tensor(
                            P[:].rearrange("p (a t) -> p a t", a=2),
                            s_w[:].rearrange("p (a t) -> p a t", a=2),
                            BC[:, L:2 * L].unsqueeze(1).broadcast_to((128, 2, L)),
                            OP.mult)
                        for hh in range(2):
                            for h in range(2):
                                nc.tensor.matmul(
                                    y_ps[2 * p_ + hh][:, h * 512:(h + 1) * 512],
                                    ident[:],
                                    P[:, hh * L + h * 512:hh * L + (h + 1) * 512],
                                    start=(n == 0), stop=(n == n_states - 1))
                # g = (u*dskip + y) * silu(z); un-flip blend -> bf16
                for cb in range(4):
                    g0 = sbl.tile([128, L], f32, name="g0", tag="g0")
                    nc.vector.scalar_tensor_tensor(
                        g0[:], u_own[cb][:], dskip_sb[:, cb:cb + 1],
                        y_ps[cb][:], OP.mult, OP.add)
                    zs = sbl.tile([128, L], f32, name="zs", tag="zs")
                    nc.scalar.activation(zs[:], z_raw[cb][:], AF.Silu)
                    g = sbl.tile([128, L], f32, name="g", tag="g")
                    nc.vector.tensor_tensor(g[:], g0[:], zs[:],
                                            OP.mult)
                    t1 = sbl.tile([128, L], f32, name="t1", tag="t1")
                    nc.vector.tensor_scalar(t1[:], g[:, ::-1],
                                            ab_sb[:, 1:2], None, OP.mult)
                    nc.vector.scalar_tensor_tensor(
                        g_send[cb][:], g[:], ab_sb[:, 0:1], t1[:],
                        OP.mult, OP.add)

            ctx_scanp.__exit__(None, None, None)

            # =========== AllToAll ===========
            send_v = send[:].rearrange("(s c r) t -> c r s t", s=NCORES, c=4)
            for cb in range(4):
                nc.sync.dma_start(
                    send_v[cb],
                    g_send[cb][:].rearrange("r (s t) -> r s t", s=NCORES))
            if single_core or skip_comm:
                nc.sync.dma_start(recv[:], send[:])
            else:
                nc.gpsimd.collective_compute(
                    "AllToAll", mybir.AluOpType.bypass,
                    replica_groups=[list(range(NCORES))],
                    ins=[send.opt()], outs=[recv.opt()])

            # =========== Phase C: tail on 2*TOK tokens ===========
            with (
                tc.tile_pool(name="tw", bufs=1) as tw,
                tc.tile_pool(name="tc_", bufs=2) as tp_,
                tc.tile_pool(name="tc_ps", bufs=2, space="PSUM") as tps,
            ):
                wout_sb = [tw.tile([128, D_MODEL], bf16, name=f"wo{k}",
                                   tag=f"wo{k}") for k in range(8)]
                for kt in range(8):
                    nc.sync.dma_start(wout_sb[kt][:],
                                      w_out_T.ap()[kt * 128:(kt + 1) * 128, :])
                fuse_sb = [tw.tile([128, 2 * D_MODEL], bf16, name=f"fu{k}",
                                   tag=f"fu{k}") for k in range(8)]
                for kt in range(8):
                    nc.sync.dma_start(fuse_sb[kt][:],
                                      fuse_w_T.ap()[kt * 128:(kt + 1) * 128, :])
                ff1_sb = [tw.tile([128, 4 * D_MODEL], bf16, name=f"f1{k}",
                                  tag=f"f1{k}") for k in range(4)]
                for kt in range(4):
                    nc.sync.dma_start(ff1_sb[kt][:],
                                      ff1_T.ap()[kt * 128:(kt + 1) * 128, :])
                ff2_sb = [tw.tile([128, D_MODEL], bf16, name=f"f2{k}",
                                  tag=f"f2{k}") for k in range(16)]
                for kt in range(16):
                    nc.sync.dma_start(ff2_sb[kt][:],
                                      ff2_T.ap()[kt * 128:(kt + 1) * 128, :])
                wno_sb = tw.tile([128, D_MODEL], f32, name="wno_sb")
                nc.sync.dma_start(wno_sb[:], w_nout_rep.ap())
                fb_sb = tw.tile([128, 8], f32, name="fb_sb")
                nc.sync.dma_start(fb_sb[:], fuse_b_col.ap())

                N2 = 2 * TOK
                gall = {}
                recv_v = recv[:].rearrange("(b q r) t -> b q r t",
                                           b=2, q=4)
                for dr in range(2):
                    for kb in range(8):
                        h, cb = kb // 4, kb % 4
                        t_ = tw.tile([128, N2], bf16, name=f"ga{dr}{kb}",
                                     tag=f"ga{dr}{kb}")
                        q = dr * 2 + h
                        src_ap = recv_v[:, q, cb * 128:(cb + 1) * 128, :]
                        nc.sync.dma_start(
                            t_[:].rearrange("r (b t) -> r b t", b=2),
                            src_ap.rearrange("b r t -> r b t"))
                        gall[(dr, kb)] = t_

                hcat = []
                for dr in range(2):
                    for mb in range(4):
                        ps = tps.tile([128, N2], f32, name="tmm", tag="tmm")
                        for kt in range(8):
                            nc.tensor.matmul(
                                ps[:],
                                wout_sb[kt][:, mb * 128:(mb + 1) * 128],
                                gall[(dr, kt)][:],
                                start=(kt == 0), stop=(kt == 7))
                        hs = tp_.tile([128, N2], bf16, name=f"hs{dr}{mb}",
                                      tag=f"hs{dr}{mb}", bufs=1)
                        nc.vector.tensor_copy(hs[:], ps[:])
                        hcat.append(hs)

                hglu = []
                sig = []
                for mb in range(4, 8):
                    ps = tps.tile([128, N2], f32, name="tmm", tag="tmm")
                    for kt in range(8):
                        nc.tensor.matmul(
                            ps[:], fuse_sb[kt][:, mb * 128:(mb + 1) * 128],
                            hcat[kt][:], start=(kt == 0), stop=(kt == 7))
                    sg = tp_.tile([128, N2], f32, name=f"sg{mb % 4}",
                                  tag=f"sg{mb % 4}", bufs=1)
                    nc.scalar.activation(sg[:], ps[:], AF.Sigmoid,
                                         bias=fb_sb[:, mb:mb + 1])
                    sig.append(sg)
                for mb in range(4):
                    ps = tps.tile([128, N2], f32, name="tmm", tag="tmm")
                    for kt in range(8):
                        nc.tensor.matmul(
                            ps[:], fuse_sb[kt][:, mb * 128:(mb + 1) * 128],
                            hcat[kt][:], start=(kt == 0), stop=(kt == 7))
                    hg = tp_.tile([128, N2], f32, name=f"hg{mb}",
                                  tag=f"hg{mb}", bufs=1)
                    nc.vector.scalar_tensor_tensor(
                        hg[:], ps[:], fb_sb[:, mb:mb + 1], sig[mb][:],
                        OP.add, OP.mult)
                    sl = tp_.tile([128, N2], bf16, name=f"sl{mb}",
                                  tag=f"sl{mb}", bufs=1)
                    nc.scalar.activation(sl[:], hg[:], AF.Silu)
                    hglu.append(sl)

                ffm = []
                for mb in range(16):
                    ps = tps.tile([128, N2], f32, name="tmm", tag="tmm")
                    for kt in range(4):
                        nc.tensor.matmul(
                            ps[:], ff1_sb[kt][:, mb * 128:(mb + 1) * 128],
                            hglu[kt][:], start=(kt == 0), stop=(kt == 3))
                    sl = tp_.tile([128, N2], bf16, name=f"fm{mb}",
                                  tag=f"fm{mb}", bufs=1)
                    nc.scalar.activation(sl[:], ps[:], AF.Silu)
                    ffm.append(sl)
                ffo = []
                for mb in range(4):
                    ps = tps.tile([128, N2], f32, name="tmm", tag="tmm")
                    for kt in range(16):
                        nc.tensor.matmul(
                            ps[:], ff2_sb[kt][:, mb * 128:(mb + 1) * 128],
                            ffm[kt][:], start=(kt == 0), stop=(kt == 15))
                    fs = tp_.tile([128, N2], f32, name=f"fo{mb}",
                                  tag=f"fo{mb}", bufs=1)
                    nc.vector.tensor_copy(fs[:], ps[:])
                    ffo.append(fs)

                for tb in range(2):
                    yt = tp_.tile([128, D_MODEL], f32, name="yt", tag="yt")
                    for db in range(4):
                        tpp = tps.tile([128, 128], f32, name="tp2", tag="tp2")
                        nc.tensor.transpose(
                            tpp[:], ffo[db][:, tb * 128:(tb + 1) * 128],
                            idf[:])
                        nc.vector.tensor_copy(
                            yt[:, db * 128:(db + 1) * 128], tpp[:])
                    xr = tp_.tile([128, D_MODEL], f32, name="xr", tag="xr")
                    nc.sync.dma_start(xr[:],
                                      x_res.ap()[tb * 128:(tb + 1) * 128, :])
                    nc.vector.tensor_tensor(yt[:], yt[:], xr[:], OP.add)
                    yn = tp_.tile([128, D_MODEL], f32, name="yn", tag="yn")
                    ssum = tp_.tile([128, 1], f32, name="ssum2", tag="ssum2")
                    nc.scalar.activation(yn[:], yt[:], AF.Square,
                                         accum_out=ssum[:])
                    lnv = tp_.tile([128, 1], f32, name="lnv2", tag="lnv2")
                    nc.scalar.activation(lnv[:], ssum[:], AF.Ln,
                                         scale=1.0 / D_MODEL, bias=c_eps[:])
                    rinv = tp_.tile([128, 1], f32, name="rinv2", tag="rinv2")
                    nc.scalar.activation(rinv[:], lnv[:], AF.Exp, scale=-0.5)
                    nc.vector.tensor_scalar(yn[:], yt[:], rinv[:], None,
                                            OP.mult)
                    yo = tp_.tile([128, D_MODEL], f32, name="yo", tag="yo")
                    nc.vector.tensor_tensor(yo[:], yn[:], wno_sb[:], OP.mult)
                    nc.sync.dma_start(out.ap()[tb * 128:(tb + 1) * 128, :],
                                      yo[:])

    nc.compile()
    return nc


def _prep_inputs(inputs):
    import ml_dtypes
    bf = ml_dtypes.bfloat16

    x = np.ascontiguousarray(np.asarray(inputs["x"], np.float32))
    W_in = np.asarray(inputs["W_in"], np.float32)
    conv_w = np.asarray(inputs["conv_w"], np.float32)[:, 0, :]
    conv_b = np.asarray(inputs["conv_b"], np.float32)
    W_xproj = np.asarray(inputs["W_xproj"], np.float32)
    W_dt = np.asarray(inputs["W_dt"], np.float32)
    b_dt = np.asarray(inputs["b_dt"], np.float32)
    A = -np.exp(np.asarray(inputs["A_log"], np.float32))
    Dskip = np.asarray(inputs["Dskip"], np.float32)
    W_out = np.asarray(inputs["W_out"], np.float32)
    norm_in_w = np.asarray(inputs["norm_in_w"], np.float32)
    fuse_W = np.asarray(inputs["fuse_W"], np.float32)
    fuse_b = np.asarray(inputs["fuse_b"], np.float32)
    ff_W1 = np.asarray(inputs["ff_W1"], np.float32)
    ff_W2 = np.asarray(inputs["ff_W2"], np.float32)
    norm_out_w = np.asarray(inputs["norm_out_w"], np.float32)

    W_in_eff = W_in * norm_in_w[None, :]
    Wu = W_in_eff[:D_INNER]
    Wz = W_in_eff[D_INNER:]

    assert np.allclose(A, A[0:1], rtol=0, atol=0), "A varies per channel"
    a_rep = np.repeat(A[0:1], 128, axis=0).astype(np.float32)

    def cols(v):
        return np.ascontiguousarray(v.reshape(4, 128).T)

    common = {
        "a_rep": a_rep,
        "ident_bf": np.eye(128, dtype=bf),
        "w_out_T": np.ascontiguousarray(W_out.T).astype(bf),
        "fuse_w_T": np.ascontiguousarray(fuse_W.T).astype(bf),
        "fuse_b_col": np.ascontiguousarray(fuse_b.reshape(8, 128).T),
        "ff1_T": np.ascontiguousarray(ff_W1.T).astype(bf),
        "ff2_T": np.ascontiguousarray(ff_W2.T).astype(bf),
        "w_nout_rep": np.repeat(norm_out_w[None, :], 128, axis=0),
    }

    maps = []
    for c in range(NCORES):
        b, dr, h = c // 4, (c % 4) // 2, c % 2
        own = slice(h * CH, (h + 1) * CH)
        perm = np.r_[np.arange(h * CH, (h + 1) * CH),
                     np.arange((1 - h) * CH, (2 - h) * CH)]

        xb_ = x[b] if dr == 0 else x[b, ::-1]
        w_in_T = np.concatenate([Wu[perm].T, Wz[own].T], axis=1)
        cw = conv_w[perm]
        convw_ = np.zeros((128, 32), np.float32)
        convb_ = np.zeros((128, 8), np.float32)
        cb_p = conv_b[perm]
        for cb in range(8):
            convw_[:, cb * 4:(cb + 1) * 4] = cw[cb * 128:(cb + 1) * 128]
            convb_[:, cb] = cb_p[cb * 128:(cb + 1) * 128]
        ab = np.zeros((128, 2), np.float32)
        ab[:, 0] = 1.0 if dr == 0 else 0.0
        ab[:, 1] = 0.0 if dr == 0 else 1.0
        tok_sl = slice(c * TOK, (c + 1) * TOK)
        x_res_ = np.concatenate([x[0, tok_sl], x[1, tok_sl]], axis=0)

        m = dict(common)
        m.update({
            "xb": np.ascontiguousarray(xb_),
            "x_res": np.ascontiguousarray(x_res_),
            "w_in_T": np.ascontiguousarray(w_in_T).astype(bf),
            "convw": convw_,
            "convb": convb_,
            "w_xproj_T": np.ascontiguousarray(W_xproj[:, perm].T).astype(bf),
            "w_dt_T": np.ascontiguousarray(W_dt[own].T).astype(bf),
            "b_dt_col": cols(b_dt[own]),
            "dskip_col": cols(Dskip[own]),
            "ab_cols": ab,
        })
        maps.append(m)
    return maps


def kernel(**inputs):
    from concourse.bass_utils import run_bass_kernel_spmd

    if "nc" not in _CACHE:
        _CACHE["nc"] = _build()
    nc = _CACHE["nc"]
    maps = _prep_inputs(inputs)
    res = run_bass_kernel_spmd(nc, maps, list(range(NCORES)))
    y = np.zeros((B, L, D_MODEL), np.float32)
    for c in range(NCORES):
        o = res.results[c]["out"]
        y[0, c * TOK:(c + 1) * TOK] = o[:TOK]
        y[1, c * TOK:(c + 1) * TOK] = o[TOK:]
    return y



# revision 26
# speedup vs baseline: 1.6456x; 1.6456x over previous
"""Trainium2 Bass kernel for nn_BiMambaBlock (B=2, L=1024, d_model=512).

Strategy (8 NeuronCores, SPMD — one identical program, per-core data):
  core c = (b, dir, half) with slot index c = b*4 + dir*2 + half.
  - dir is handled by feeding bwd cores time-flipped x; the whole Mamba
    pipeline runs in "physical" (possibly flipped) time. A data-driven
    blend (alpha,beta in {0,1} per core) un-flips the gated output g for
    bwd cores, so the program has zero direction-dependent control flow.
  - Channel halves: the host permutes the in-proj weight columns so the
    core's OWN 512 channels are always u-blocks 0..3; matching row
    permutations are applied to W_xproj / conv weights.
  - Each core computes: rmsnorm -> in-proj (full u for x_dbl, own z half)
    -> causal conv -> x_dbl -> dt/B/C -> selective scan on its 512
    channels -> gated g -> un-flip blend -> 8-way AllToAll (token
    sharding, bf16) -> tail (out-proj, GLU fuse, FF, residual, out-norm)
    for its 128-token slice of BOTH batches.
Scan: channels in partitions, time in the free dim; n-outer loop over the
64 state indices. Per n: B_n|C_n row is replicated to 128 partitions by a
broadcast DMA (DRAM row with partition-stride-0 source); dA = exp(a_n*dt)
on ScalarE (bf16); dBu and the C-multiply are bf16 tensor_tensor ops
statically load-balanced between the DVE and the GPSIMD (Pool) engine;
the recurrence runs on the DVE tensor_tensor_scan; y is accumulated over
n with bf16 identity matmuls into PSUM (fp32). Tail weights are
prefetched during phase A / the scan.
"""

import numpy as np

D_MODEL = 512
D_STATE = 64
D_CONV = 4
D_INNER = 1024
DT_RANK = 32
B = 2
L = 1024
EPS = 1e-6
NCORES = 8
CH = 512
TOK = L // NCORES

# smalls packing offsets (f32 [128, SMALLS_K])
OFF_CONVW = 0          # [128, 32]
OFF_CONVB = 32         # [128, 8]
OFF_BDT = 40           # [128, 4]
OFF_DSKIP = 44         # [128, 4]
OFF_AB = 48            # [128, 2]
OFF_AREP = 50          # [128, 64]
OFF_FUSEB = 114        # [128, 8]
OFF_WNOUT = 122        # [128, 512]
SMALLS_K = 634

POOL_MULTS = 55        # of 128 scan multiplies routed to GPSIMD (Pool)

_CACHE = {}


def _build(single_core=False, skip_scan=False, skip_comm=False,
           pool_mults=POOL_MULTS):
    import concourse.bacc as bacc
    import concourse.mybir as mybir
    import concourse.tile as tile
    from concourse.masks import make_identity

    f32 = mybir.dt.float32
    bf16 = mybir.dt.bfloat16
    AF = mybir.ActivationFunctionType
    OP = mybir.AluOpType

    nc = bacc.Bacc("TRN2", target_bir_lowering=False, debug=False,
                   num_devices=1 if single_core else NCORES)

    def din(name, shape, dt_=f32):
        return nc.dram_tensor(name, shape, dt_, kind="ExternalInput")

    xb = din("xb", [L, D_MODEL], bf16)
    x_res = din("x_res", [2 * TOK, D_MODEL])
    w_in_T = din("w_in_T", [D_MODEL, D_INNER + CH], bf16)
    w_xproj_T = din("w_xproj_T", [D_INNER, DT_RANK + 2 * D_STATE], bf16)
    w_dt_T = din("w_dt_T", [DT_RANK, CH], bf16)
    w_out_T = din("w_out_T", [D_INNER, D_MODEL], bf16)
    fuse_w_T = din("fuse_w_T", [2 * D_MODEL, 2 * D_MODEL], bf16)
    ff1_T = din("ff1_T", [D_MODEL, 4 * D_MODEL], bf16)
    ff2_T = din("ff2_T", [4 * D_MODEL, D_MODEL], bf16)
    smalls = din("smalls", [128, SMALLS_K])
    out = nc.dram_tensor("out", [2 * TOK, D_MODEL], f32, kind="ExternalOutput")

    # which of the 128 scan multiplies ([128,4L] each: 64 dBu + 64 P,
    # interleaved as 2n / 2n+1) run on GPSIMD
    on_pool = [((i + 1) * pool_mults) // 128 > (i * pool_mults) // 128
               for i in range(128)]

    with tile.TileContext(nc) as tc:
        with (
            tc.tile_pool(name="wpool", bufs=1) as wp,
            tc.tile_pool(name="actp", bufs=1) as actp,
            tc.tile_pool(name="tw", bufs=1) as tw,
            tc.tile_pool(name="dram", bufs=1, space="DRAM") as dramp,
        ):
            sm_sb = wp.tile([128, SMALLS_K], f32, name="sm_sb")
            nc.sync.dma_start(sm_sb[:], smalls.ap())
            c_eps = wp.tile([128, 1], f32, name="c_eps")
            nc.vector.memset(c_eps[:], EPS)
            c_one = wp.tile([128, 1], f32, name="c_one")
            nc.vector.memset(c_one[:], 1.0)
            idf = wp.tile([128, 128], f32, name="idf")
            make_identity(nc, idf[:])
            ident = wp.tile([128, 128], bf16, name="ident")
            nc.vector.tensor_copy(ident[:], idf[:])

            # tail weights — prefetch immediately, consumed in phase C
            wout_sb = [tw.tile([128, D_MODEL], bf16, name=f"wo{k}",
                               tag=f"wo{k}") for k in range(8)]
            for kt in range(8):
                nc.sync.dma_start(wout_sb[kt][:],
                                  w_out_T.ap()[kt * 128:(kt + 1) * 128, :])
            fuse_sb = [tw.tile([128, 2 * D_MODEL], bf16, name=f"fu{k}",
                               tag=f"fu{k}") for k in range(8)]
            for kt in range(8):
                nc.sync.dma_start(fuse_sb[kt][:],
                                  fuse_w_T.ap()[kt * 128:(kt + 1) * 128, :])
            ff1_sb = [tw.tile([128, 4 * D_MODEL], bf16, name=f"f1{k}",
                              tag=f"f1{k}") for k in range(4)]
            for kt in range(4):
                nc.sync.dma_start(ff1_sb[kt][:],
                                  ff1_T.ap()[kt * 128:(kt + 1) * 128, :])
            ff2_sb = [tw.tile([128, D_MODEL], bf16, name=f"f2{k}",
                              tag=f"f2{k}") for k in range(16)]

            send = dramp.tile([NCORES * CH, TOK], bf16, name="sendbuf")
            recv = dramp.tile([NCORES * CH, TOK], bf16, name="recvbuf")
            bc_dram = dramp.tile([D_STATE, 2 * L], bf16, name="bc_dram")

            g_send = [actp.tile([128, L], bf16, name=f"gs{i}", tag=f"gs{i}")
                      for i in range(4)]

            # =========== Phase A / B (scan-lifetime pool) ===========
            ctx_scanp = tc.tile_pool(name="scanp", bufs=1)
            scanp = ctx_scanp.__enter__()
            z_raw = [scanp.tile([128, L], bf16, name=f"z{i}", tag=f"z{i}")
                     for i in range(4)]
            dtw_all = scanp.tile([128, 4 * L], bf16, name="dtw_all")
            dtu_all = scanp.tile([128, 4 * L], bf16, name="dtu_all")
            u_bf = [scanp.tile([128, L], bf16, name=f"ub{i}", tag=f"ub{i}")
                    for i in range(4)]

            with (
                tc.tile_pool(name="uop", bufs=1) as uop,
                tc.tile_pool(name="wxw", bufs=1) as wxw,
                tc.tile_pool(name="pa", bufs=2) as pa,
                tc.tile_pool(name="pa_ps", bufs=2, space="PSUM") as pa_ps,
                tc.tile_pool(name="upp", bufs=1) as upp,
            ):
                wxp_sb = [wxw.tile([128, 160], bf16, name=f"wx{k}", tag=f"wx{k}")
                          for k in range(8)]
                for kt in range(8):
                    nc.sync.dma_start(wxp_sb[kt][:],
                                      w_xproj_T.ap()[kt * 128:(kt + 1) * 128, :])
                wdt_sb = wxw.tile([DT_RANK, CH], bf16, name="wdt_sb")
                nc.sync.dma_start(wdt_sb[:], w_dt_T.ap())
                dtr_sb = wxw.tile([32, L], bf16, name="dtr_sb")
                B_sb = wxw.tile([64, L], bf16, name="B_sb")
                C_sb = wxw.tile([64, L], bf16, name="C_sb")

                ip_ctx = tc.tile_pool(name="ipw", bufs=1)
                ipw = ip_ctx.__enter__()
                winT_sb = [ipw.tile([128, D_INNER + CH], bf16,
                                    name=f"wi{k}", tag=f"wi{k}")
                           for k in range(4)]
                for kt in range(4):
                    nc.sync.dma_start(winT_sb[kt][:],
                                      w_in_T.ap()[kt * 128:(kt + 1) * 128, :])

                # rmsnorm + transpose -> hT (bf16) [4][128, L]
                # (activation stages batched by function to avoid act-table
                # reloads: 8x Square, then one Ln, one Exp)
                hT = [ipw.tile([128, L], bf16, name=f"hT{i}", tag=f"hT{i}")
                      for i in range(4)]
                xts = [pa.tile([128, D_MODEL], bf16, name=f"xt{tb}",
                               tag=f"xt{tb}", bufs=1) for tb in range(8)]
                ssum_all = pa.tile([128, 8], f32, name="ssum_all", bufs=1)
                rinv_all = pa.tile([128, 8], f32, name="rinv_all", bufs=1)
                for tb in range(8):
                    nc.sync.dma_start(xts[tb][:],
                                      xb.ap()[tb * 128:(tb + 1) * 128, :])
                for tb in range(8):
                    sq = pa.tile([128, D_MODEL], bf16, name="sq", tag="sq")
                    nc.scalar.activation(sq[:], xts[tb][:], AF.Square,
                                         accum_out=ssum_all[:, tb:tb + 1])
                lnv = pa.tile([128, 8], f32, name="lnv", tag="lnv", bufs=1)
                nc.scalar.activation(lnv[:], ssum_all[:], AF.Ln,
                                     scale=1.0 / D_MODEL, bias=c_eps[:])
                nc.scalar.activation(rinv_all[:], lnv[:], AF.Exp, scale=-0.5)
                for tb in range(8):
                    hn = pa.tile([128, D_MODEL], bf16, name="hn", tag="hn")
                    nc.vector.tensor_scalar(hn[:], xts[tb][:],
                                            rinv_all[:, tb:tb + 1], None,
                                            OP.mult)
                    for db in range(4):
                        tp = pa_ps.tile([128, 128], bf16, name="tp", tag="tp")
                        nc.tensor.transpose(tp[:],
                                            hn[:, db * 128:(db + 1) * 128],
                                            ident[:])
                        nc.vector.tensor_copy(
                            hT[db][:, tb * 128:(tb + 1) * 128], tp[:])

                # in-proj -> u_pre (bf16, full Din) and z (own half)
                u_pre = [upp.tile([128, L], bf16, name=f"up{i}", tag=f"up{i}")
                         for i in range(8)]
                for mb in range(12):
                    for nh in range(2):
                        ps = pa_ps.tile([128, 512], f32, name="mm", tag="mm")
                        for kt in range(4):
                            nc.tensor.matmul(
                                ps[:],
                                winT_sb[kt][:, mb * 128:(mb + 1) * 128],
                                hT[kt][:, nh * 512:(nh + 1) * 512],
                                start=(kt == 0), stop=(kt == 3))
                        if mb < 8:
                            nc.scalar.activation(
                                u_pre[mb][:, nh * 512:(nh + 1) * 512], ps[:],
                                AF.Copy)
                        else:
                            zb = mb - 8
                            nc.scalar.activation(
                                z_raw[zb][:, nh * 512:(nh + 1) * 512], ps[:],
                                AF.Copy)
                ip_ctx.__exit__(None, None, None)

                # causal conv (zero left pad via shrinking write ranges)
                u_bf8 = list(u_bf) + [
                    upp.tile([128, L], bf16, name=f"ubx{i}", tag=f"ubx{i}")
                    for i in range(4)]
                KC = D_CONV - 1
                for cb in range(8):
                    eng = nc.vector
                    uc = pa.tile([128, L], bf16, name="uc", tag="uc", bufs=2)
                    eng.tensor_scalar(
                        uc[:], u_pre[cb][:],
                        sm_sb[:, OFF_CONVW + cb * 4 + KC:
                              OFF_CONVW + cb * 4 + KC + 1], None,
                        OP.mult)
                    for k in range(KC):
                        sh = KC - k
                        eng.scalar_tensor_tensor(
                            uc[:, sh:L], u_pre[cb][:, 0:L - sh],
                            sm_sb[:, OFF_CONVW + cb * 4 + k:
                                  OFF_CONVW + cb * 4 + k + 1],
                            uc[:, sh:L], OP.mult, OP.add)
                    nc.scalar.activation(
                        u_bf8[cb][:], uc[:], AF.Silu,
                        bias=sm_sb[:, OFF_CONVB + cb:OFF_CONVB + cb + 1])

                # x_dbl -> dtr (bf16), B, C (bf16)
                for nh in range(2):
                    ps0 = pa_ps.tile([32, 512], f32, name="mm32", tag="mm32",
                                     bufs=1)
                    ps1 = pa_ps.tile([64, 512], f32, name="mmB", tag="mmB",
                                     bufs=1)
                    ps2 = pa_ps.tile([64, 512], f32, name="mmC", tag="mmC",
                                     bufs=1)
                    for kt in range(8):
                        nc.tensor.matmul(
                            ps0[:], wxp_sb[kt][:, 0:32],
                            u_bf8[kt][:, nh * 512:(nh + 1) * 512],
                            start=(kt == 0), stop=(kt == 7))
                    for kt in range(8):
                        nc.tensor.matmul(
                            ps1[:], wxp_sb[kt][:, 32:96],
                            u_bf8[kt][:, nh * 512:(nh + 1) * 512],
                            start=(kt == 0), stop=(kt == 7))
                    for kt in range(8):
                        nc.tensor.matmul(
                            ps2[:], wxp_sb[kt][:, 96:160],
                            u_bf8[kt][:, nh * 512:(nh + 1) * 512],
                            start=(kt == 0), stop=(kt == 7))
                    nc.scalar.activation(dtr_sb[:, nh * 512:(nh + 1) * 512],
                                         ps0[:], AF.Copy)
                    nc.scalar.activation(B_sb[:, nh * 512:(nh + 1) * 512],
                                         ps1[:], AF.Copy)
                    nc.scalar.activation(C_sb[:, nh * 512:(nh + 1) * 512],
                                         ps2[:], AF.Copy)

                # B/C -> [n, B_n|C_n] DRAM rows for broadcast reads
                nc.sync.dma_start(bc_dram[:, 0:L], B_sb[:])
                nc.sync.dma_start(bc_dram[:, L:2 * L], C_sb[:])

                # dt = softplus(dtr @ W_dt^T + b_dt); dtu = dt * u
                # (all 8 Exp then all 8 Ln — two act-table loads, not 16)
                exs = [pa.tile([128, 512], bf16, name=f"ex{i}",
                               tag=f"ex{i}", bufs=1) for i in range(8)]
                for mb in range(4):
                    for nh in range(2):
                        ps = pa_ps.tile([128, 512], f32, name="mm", tag="mm")
                        nc.tensor.matmul(
                            ps[:], wdt_sb[:, mb * 128:(mb + 1) * 128],
                            dtr_sb[:, nh * 512:(nh + 1) * 512],
                            start=True, stop=True)
                        nc.scalar.activation(
                            exs[mb * 2 + nh][:], ps[:], AF.Exp,
                            bias=sm_sb[:, OFF_BDT + mb:OFF_BDT + mb + 1])
                for mb in range(4):
                    for nh in range(2):
                        off = mb * L + nh * 512
                        nc.scalar.activation(
                            dtw_all[:, off:off + 512],
                            exs[mb * 2 + nh][:], AF.Ln, bias=c_one[:])
                for cb in range(4):
                    nc.vector.tensor_tensor(
                        dtu_all[:, cb * L:(cb + 1) * L],
                        dtw_all[:, cb * L:(cb + 1) * L],
                        u_bf[cb][:], OP.mult)

            # =========== Phase B: the scan (n outer) ===========
            with (
                tc.tile_pool(name="sb_ps", bufs=1, space="PSUM") as sb_ps,
                tc.tile_pool(name="sbl", bufs=2) as sbl,
            ):
                y_ps = [sb_ps.tile([128, L], f32, name=f"y{cb}", tag=f"y{cb}")
                        for cb in range(4)]
                n_states = 1 if skip_scan else D_STATE
                # whole-n engine assignment: a "pool n" runs both its
                # multiplies on GPSIMD; scans always run on DVE.  Pool n's
                # are software-pipelined one deep against the DVE stream.
                n_pool = 0 if skip_scan else (pool_mults * n_states) // 128
                is_pool_n = [((i + 1) * n_pool) // n_states
                             > (i * n_pool) // n_states
                             for i in range(n_states)]

                def emit_start(n, pool):
                    s = "p" if pool else "d"
                    BC = sbl.tile([128, 2 * L], bf16, name="BC",
                                  tag=f"BC{s}")
                    nc.sync.dma_start(
                        BC[:],
                        bc_dram[n:n + 1, :].broadcast_to((128, 2 * L)))
                    dA = sbl.tile([128, 4 * L], bf16, name="dA",
                                  tag=f"dA{s}")
                    nc.scalar.activation(
                        dA[:], dtw_all[:], AF.Exp,
                        scale=sm_sb[:, OFF_AREP + n:OFF_AREP + n + 1])
                    dBu = sbl.tile([128, 4 * L], bf16, name="dBu",
                                   tag=f"dBu{s}")
                    eng = nc.gpsimd if pool else nc.vector
                    eng.tensor_tensor(
                        dBu[:].rearrange("p (a t) -> p a t", a=4),
                        dtu_all[:].rearrange("p (a t) -> p a t", a=4),
                        BC[:, 0:L].unsqueeze(1).broadcast_to((128, 4, L)),
                        OP.mult)
                    return BC, dA, dBu

                cnt = [0]

                def emit_finish(tiles, pool):
                    BC, dA, dBu = tiles
                    # in-place scan: state overwrites dBu
                    for cb in range(4):
                        nc.vector.tensor_tensor_scan(
                            dBu[:, cb * L:(cb + 1) * L],
                            dA[:, cb * L:(cb + 1) * L],
                            dBu[:, cb * L:(cb + 1) * L],
                            0.0, OP.mult, OP.add)
                    # in-place C-multiply: P overwrites dA
                    eng = nc.gpsimd if pool else nc.vector
                    eng.tensor_tensor(
                        dA[:].rearrange("p (a t) -> p a t", a=4),
                        dBu[:].rearrange("p (a t) -> p a t", a=4),
                        BC[:, L:2 * L].unsqueeze(1).broadcast_to((128, 4, L)),
                        OP.mult)
                    first, last = cnt[0] == 0, cnt[0] == n_states - 1
                    for cb in range(4):
                        for h in range(2):
                            nc.tensor.matmul(
                                y_ps[cb][:, h * 512:(h + 1) * 512],
                                ident[:],
                                dA[:, cb * L + h * 512:
                                   cb * L + (h + 1) * 512],
                                start=first, stop=last)
                    cnt[0] += 1

                pending = []
                for n in range(n_states):
                    if is_pool_n[n]:
                        if len(pending) == 2:
                            emit_finish(pending.pop(0), True)
                        pending.append(emit_start(n, True))
                    else:
                        tiles = emit_start(n, False)
                        emit_finish(tiles, False)
                while pending:
                    emit_finish(pending.pop(0), True)
                # g = (u*dskip + y) * silu(z); un-flip blend -> bf16
                for cb in range(4):
                    g0 = sbl.tile([128, L], bf16, name="g0", tag="g0",
                                  bufs=1)
                    nc.vector.scalar_tensor_tensor(
                        g0[:], u_bf[cb][:],
                        sm_sb[:, OFF_DSKIP + cb:OFF_DSKIP + cb + 1],
                        y_ps[cb][:], OP.mult, OP.add)
                    zs = sbl.tile([128, L], bf16, name="zs", tag="zs",
                                  bufs=1)
                    nc.scalar.activation(zs[:], z_raw[cb][:], AF.Silu)
                    g = sbl.tile([128, L], bf16, name="g", tag="g", bufs=1)
                    nc.vector.tensor_tensor(g[:], g0[:], zs[:],
                                            OP.mult)
                    t1 = sbl.tile([128, L], bf16, name="t1", tag="t1",
                                  bufs=1)
                    nc.vector.tensor_scalar(
                        t1[:], g[:, ::-1],
                        sm_sb[:, OFF_AB + 1:OFF_AB + 2], None, OP.mult)
                    nc.vector.scalar_tensor_tensor(
                        g_send[cb][:], g[:],
                        sm_sb[:, OFF_AB:OFF_AB + 1], t1[:],
                        OP.mult, OP.add)

            ctx_scanp.__exit__(None, None, None)

            # =========== AllToAll ===========
            send_v = send[:].rearrange("(s c r) t -> c r s t", s=NCORES, c=4)
            for cb in range(4):
                nc.sync.dma_start(
                    send_v[cb],
                    g_send[cb][:].rearrange("r (s t) -> r s t", s=NCORES))
            if single_core or skip_comm:
                nc.sync.dma_start(recv[:], send[:])
            else:
                nc.gpsimd.collective_compute(
                    "AllToAll", mybir.AluOpType.bypass,
                    replica_groups=[list(range(NCORES))],
                    ins=[send.opt()], outs=[recv.opt()])
            # ff2 loads deferred to overlap the collective
            for kt in range(16):
                nc.sync.dma_start(ff2_sb[kt][:],
                                  ff2_T.ap()[kt * 128:(kt + 1) * 128, :])

            # =========== Phase C: tail on 2*TOK tokens ===========
            with (
                tc.tile_pool(name="tc_", bufs=2) as tp_,
                tc.tile_pool(name="tc_ps", bufs=4, space="PSUM") as tps,
            ):
                N2 = 2 * TOK
                gall = {}
                recv_v = recv[:].rearrange("(b q r) t -> b q r t",
                                           b=2, q=4)
                for dr in range(2):
                    for kb in range(8):
                        h, cb = kb // 4, kb % 4
                        t_ = tp_.tile([128, N2], bf16, name=f"ga{dr}{kb}",
                                      tag=f"ga{dr}{kb}", bufs=1)
                        q = dr * 2 + h
                        src_ap = recv_v[:, q, cb * 128:(cb + 1) * 128, :]
                        nc.sync.dma_start(
                            t_[:].rearrange("r (b t) -> r b t", b=2),
                            src_ap.rearrange("b r t -> r b t"))
                        gall[(dr, kb)] = t_

                hcat = []
                for dr in range(2):
                    for mb in range(4):
                        ps = tps.tile([128, N2], f32, name="tmm", tag="tmm")
                        for kt in range(8):
                            nc.tensor.matmul(
                                ps[:],
                                wout_sb[kt][:, mb * 128:(mb + 1) * 128],
                                gall[(dr, kt)][:],
                                start=(kt == 0), stop=(kt == 7))
                        hs = tp_.tile([128, N2], bf16, name=f"hs{dr}{mb}",
                                      tag=f"hs{dr}{mb}", bufs=1)
                        nc.vector.tensor_copy(hs[:], ps[:])
                        hcat.append(hs)

                hglu = []
                sig = []
                for mb in range(4, 8):
                    ps = tps.tile([128, N2], f32, name="tmm", tag="tmm")
                    for kt in range(8):
                        nc.tensor.matmul(
                            ps[:], fuse_sb[kt][:, mb * 128:(mb + 1) * 128],
                            hcat[kt][:], start=(kt == 0), stop=(kt == 7))
                    sg = tp_.tile([128, N2], f32, name=f"sg{mb % 4}",
                                  tag=f"sg{mb % 4}", bufs=1)
                    nc.scalar.activation(
                        sg[:], ps[:], AF.Sigmoid,
                        bias=sm_sb[:, OFF_FUSEB + mb:OFF_FUSEB + mb + 1])
                    sig.append(sg)
                for mb in range(4):
                    ps = tps.tile([128, N2], f32, name="tmm", tag="tmm")
                    for kt in range(8):
                        nc.tensor.matmul(
                            ps[:], fuse_sb[kt][:, mb * 128:(mb + 1) * 128],
                            hcat[kt][:], start=(kt == 0), stop=(kt == 7))
                    hg = tp_.tile([128, N2], f32, name=f"hg{mb}",
                                  tag=f"hg{mb}", bufs=1)
                    nc.vector.scalar_tensor_tensor(
                        hg[:], ps[:],
                        sm_sb[:, OFF_FUSEB + mb:OFF_FUSEB + mb + 1],
                        sig[mb][:], OP.add, OP.mult)
                    sl = tp_.tile([128, N2], bf16, name=f"sl{mb}",
                                  tag=f"sl{mb}", bufs=1)
                    nc.scalar.activation(sl[:], hg[:], AF.Silu)
                    hglu.append(sl)

                ffm = []
                for mb in range(16):
                    ps = tps.tile([128, N2], f32, name="tmm", tag="tmm")
                    for kt in range(4):
                        nc.tensor.matmul(
                            ps[:], ff1_sb[kt][:, mb * 128:(mb + 1) * 128],
                            hglu[kt][:], start=(kt == 0), stop=(kt == 3))
                    sl = tp_.tile([128, N2], bf16, name=f"fm{mb}",
                                  tag=f"fm{mb}", bufs=1)
                    nc.scalar.activation(sl[:], ps[:], AF.Silu)
                    ffm.append(sl)
                ffo = []
                for mb in range(4):
                    ps = tps.tile([128, N2], f32, name="tmm", tag="tmm")
                    for kt in range(16):
                        nc.tensor.matmul(
                            ps[:], ff2_sb[kt][:, mb * 128:(mb + 1) * 128],
                            ffm[kt][:], start=(kt == 0), stop=(kt == 15))
                    fs = tp_.tile([128, N2], f32, name=f"fo{mb}",
                                  tag=f"fo{mb}", bufs=1)
                    nc.vector.tensor_copy(fs[:], ps[:])
                    ffo.append(fs)

                for tb in range(2):
                    yt = tp_.tile([128, D_MODEL], f32, name="yt", tag="yt")
                    for db in range(4):
                        tpp = tps.tile([128, 128], f32, name="tp2", tag="tp2")
                        nc.tensor.transpose(
                            tpp[:], ffo[db][:, tb * 128:(tb + 1) * 128],
                            idf[:])
                        nc.vector.tensor_copy(
                            yt[:, db * 128:(db + 1) * 128], tpp[:])
                    xr = tp_.tile([128, D_MODEL], f32, name="xr", tag="xr")
                    nc.sync.dma_start(xr[:],
                                      x_res.ap()[tb * 128:(tb + 1) * 128, :])
                    nc.vector.tensor_tensor(yt[:], yt[:], xr[:], OP.add)
                    yn = tp_.tile([128, D_MODEL], f32, name="yn", tag="yn")
                    ssum = tp_.tile([128, 1], f32, name="ssum2", tag="ssum2")
                    nc.scalar.activation(yn[:], yt[:], AF.Square,
                                         accum_out=ssum[:])
                    lnv = tp_.tile([128, 1], f32, name="lnv2", tag="lnv2")
                    nc.scalar.activation(lnv[:], ssum[:], AF.Ln,
                                         scale=1.0 / D_MODEL, bias=c_eps[:])
                    rinv = tp_.tile([128, 1], f32, name="rinv2", tag="rinv2")
                    nc.scalar.activation(rinv[:], lnv[:], AF.Exp, scale=-0.5)
                    nc.vector.tensor_scalar(yn[:], yt[:], rinv[:], None,
                                            OP.mult)
                    yo = tp_.tile([128, D_MODEL], f32, name="yo", tag="yo")
                    nc.vector.tensor_tensor(
                        yo[:], yn[:],
                        sm_sb[:, OFF_WNOUT:OFF_WNOUT + D_MODEL], OP.mult)
                    nc.sync.dma_start(out.ap()[tb * 128:(tb + 1) * 128, :],
                                      yo[:])

    nc.compile()
    return nc


def _prep_inputs(inputs):
    import ml_dtypes
    bf = ml_dtypes.bfloat16

    x = np.ascontiguousarray(np.asarray(inputs["x"], np.float32))
    W_in = np.asarray(inputs["W_in"], np.float32)
    conv_w = np.asarray(inputs["conv_w"], np.float32)[:, 0, :]
    conv_b = np.asarray(inputs["conv_b"], np.float32)
    W_xproj = np.asarray(inputs["W_xproj"], np.float32)
    W_dt = np.asarray(inputs["W_dt"], np.float32)
    b_dt = np.asarray(inputs["b_dt"], np.float32)
    A = -np.exp(np.asarray(inputs["A_log"], np.float32))
    Dskip = np.asarray(inputs["Dskip"], np.float32)
    W_out = np.asarray(inputs["W_out"], np.float32)
    norm_in_w = np.asarray(inputs["norm_in_w"], np.float32)
    fuse_W = np.asarray(inputs["fuse_W"], np.float32)
    fuse_b = np.asarray(inputs["fuse_b"], np.float32)
    ff_W1 = np.asarray(inputs["ff_W1"], np.float32)
    ff_W2 = np.asarray(inputs["ff_W2"], np.float32)
    norm_out_w = np.asarray(inputs["norm_out_w"], np.float32)

    W_in_eff = W_in * norm_in_w[None, :]
    Wu = W_in_eff[:D_INNER]
    Wz = W_in_eff[D_INNER:]

    assert np.allclose(A, A[0:1], rtol=0, atol=0), "A varies per channel"
    a_rep = np.repeat(A[0:1], 128, axis=0).astype(np.float32)

    def cols(v):
        return np.ascontiguousarray(v.reshape(4, 128).T)

    common = {
        "w_out_T": np.ascontiguousarray(W_out.T).astype(bf),
        "fuse_w_T": np.ascontiguousarray(fuse_W.T).astype(bf),
        "ff1_T": np.ascontiguousarray(ff_W1.T).astype(bf),
        "ff2_T": np.ascontiguousarray(ff_W2.T).astype(bf),
    }

    maps = []
    for c in range(NCORES):
        b, dr, h = c // 4, (c % 4) // 2, c % 2
        own = slice(h * CH, (h + 1) * CH)
        perm = np.r_[np.arange(h * CH, (h + 1) * CH),
                     np.arange((1 - h) * CH, (2 - h) * CH)]

        xb_ = x[b] if dr == 0 else x[b, ::-1]
        w_in_T = np.concatenate([Wu[perm].T, Wz[own].T], axis=1)
        cw = conv_w[perm]
        sm = np.zeros((128, SMALLS_K), np.float32)
        cb_p = conv_b[perm]
        for cb in range(8):
            sm[:, OFF_CONVW + cb * 4:OFF_CONVW + (cb + 1) * 4] = \
                cw[cb * 128:(cb + 1) * 128]
            sm[:, OFF_CONVB + cb] = cb_p[cb * 128:(cb + 1) * 128]
        sm[:, OFF_BDT:OFF_BDT + 4] = cols(b_dt[own])
        sm[:, OFF_DSKIP:OFF_DSKIP + 4] = cols(Dskip[own])
        sm[:, OFF_AB] = 1.0 if dr == 0 else 0.0
        sm[:, OFF_AB + 1] = 0.0 if dr == 0 else 1.0
        sm[:, OFF_AREP:OFF_AREP + D_STATE] = a_rep
        sm[:, OFF_FUSEB:OFF_FUSEB + 8] = \
            np.ascontiguousarray(fuse_b.reshape(8, 128).T)
        sm[:, OFF_WNOUT:OFF_WNOUT + D_MODEL] = \
            np.repeat(norm_out_w[None, :], 128, axis=0)
        tok_sl = slice(c * TOK, (c + 1) * TOK)
        x_res_ = np.concatenate([x[0, tok_sl], x[1, tok_sl]], axis=0)

        m = dict(common)
        m.update({
            "xb": np.ascontiguousarray(xb_).astype(bf),
            "x_res": np.ascontiguousarray(x_res_),
            "w_in_T": np.ascontiguousarray(w_in_T).astype(bf),
            "w_xproj_T": np.ascontiguousarray(W_xproj[:, perm].T).astype(bf),
            "w_dt_T": np.ascontiguousarray(W_dt[own].T).astype(bf),
            "smalls": sm,
        })
        maps.append(m)
    return maps


def kernel(**inputs):
    from concourse.bass_utils import run_bass_kernel_spmd

    if "nc" not in _CACHE:
        _CACHE["nc"] = _build()
    nc = _CACHE["nc"]
    maps = _prep_inputs(inputs)
    res = run_bass_kernel_spmd(nc, maps, list(range(NCORES)))
    y = np.zeros((B, L, D_MODEL), np.float32)
    for c in range(NCORES):
        o = res.results[c]["out"]
        y[0, c * TOK:(c + 1) * TOK] = o[:TOK]
        y[1, c * TOK:(c + 1) * TOK] = o[TOK:]
    return y
